# revision 2
# baseline (speedup 1.0000x reference)
# Trainium2 Bass kernel for the non-local attention block (nn_DRAL_88476326297980).
#
# Reference computation (per batch b):
#   theta = theta_w @ x_b + theta_b            (CI=128, N=4096)
#   phi   = maxpool2x2(phi_w @ y_b + phi_b)    (CI=128, P=1024)
#   g     = maxpool2x2(g_w  @ y_b + g_b)       (CI=128, P=1024)
#   f     = theta^T @ phi                      (N, P)
#   fdiv  = softmax(f, axis=P)
#   z     = fdiv @ g^T                         (N, CI)
#   wz    = W_w @ z^T + W_b                    (C=256, N)
#   out   = BN(wz over all b,n) + x            (training-mode batch stats)
#
# Sharding: data-parallel over batch, 2 batches per core, 8 cores.
# BN batch statistics are combined with a tiny (128x4) AllReduce.
#
# Math simplifications used (exact, not approximations):
#  - phi_b adds a per-row constant to f -> softmax-invariant -> dropped.
#  - g_b adds a per-CI constant to z (softmax weights sum to 1) -> shifts wz
#    per-channel -> cancelled by the BN mean subtraction -> dropped.
#  - W_b shifts wz per-channel -> cancelled by BN mean subtraction -> dropped.
#  - sum(wz^2) for the BN variance is computed on the PE as diag(W ZZ W^T)
#    where ZZ = sum_m z_m z_m^T is a Gram matrix accumulated in PSUM, instead
#    of an elementwise square+reduce pass over wz.
#
# Layout choices:
#  - f is computed TRANSPOSED (fT: pooled dim on partitions, n on free) so both
#    attention matmuls contract over the partition dim with no transposes of f.
#  - softmax denominators come from an extra all-ones column appended to the
#    g^T tiles (padded to 256 cols so float32r matmuls run at full rate): the
#    z-matmul then yields [z_unnorm | s | 0...] in one PSUM accumulation.
#  - z tiles are normalized with a broadcast reciprocal multiply, transposed
#    on the PE (128x128) to give zT (CI on partitions) for the W conv.
#  - all matmul operands use float32r (fp32 rounded to 11-bit mantissa, full
#    PE rate at N>=256); producers (DMA/ACT/DVE) write float32r directly.

import numpy as np
from ml_dtypes import bfloat16 as ml_bf16

import concourse.bass as bass
import concourse.mybir as mybir
import concourse.tile as tile
from concourse import bacc
from concourse.bass_utils import run_bass_kernel_spmd

F32 = mybir.dt.float32
F32R = mybir.dt.float32r
BF16 = mybir.dt.bfloat16
ALU = mybir.AluOpType
ACT = mybir.ActivationFunctionType
AX = mybir.AxisListType

NCORES = 8
B = 16
BLOC = B // NCORES          # 2 batches per core
C = 256                     # in channels
CI = 128                    # inter channels
N = 4096                    # h*w
MC = 512                    # m-chunk (columns per matmul)
NMC = N // MC               # 8
EPS = 1e-5
COUNT = B * N               # BN sample count per channel


def build_body(tc, io):
    nc = tc.nc
    x, y, wpack, vpack, gpad, out = (
        io["x"], io["y"], io["wpack"], io["vpack"], io["gpad"], io["out"],
    )

    ctx = io["ctx"]
    consts = ctx.enter_context(tc.tile_pool(name="consts", bufs=1))
    xfp = ctx.enter_context(tc.tile_pool(name="xfp", bufs=2))
    yin = ctx.enter_context(tc.tile_pool(name="yin", bufs=3))
    thp = ctx.enter_context(tc.tile_pool(name="thp", bufs=1))
    poolp = ctx.enter_context(tc.tile_pool(name="poolp", bufs=1))
    ptmp = ctx.enter_context(tc.tile_pool(name="ptmp", bufs=1))
    gtp = ctx.enter_context(tc.tile_pool(name="gtp", bufs=2))
    fxp = ctx.enter_context(tc.tile_pool(name="fxp", bufs=2))
    znp = ctx.enter_context(tc.tile_pool(name="znp", bufs=6))
    ztp = ctx.enter_context(tc.tile_pool(name="ztp", bufs=1))
    wzp = ctx.enter_context(tc.tile_pool(name="wzp", bufs=4))
    outp = ctx.enter_context(tc.tile_pool(name="outp", bufs=4))
    psf = ctx.enter_context(tc.tile_pool(name="psf", bufs=2, space="PSUM"))
    pbank = ctx.enter_context(tc.tile_pool(name="pbank", bufs=3, space="PSUM"))
    pzz = ctx.enter_context(tc.tile_pool(name="pzz", bufs=1, space="PSUM"))
    dram = ctx.enter_context(tc.tile_pool(name="dram", bufs=1, space="DRAM"))

    # ---- constants / weights: two packed DMAs to keep sync fan-in tiny ----
    # wpack (128, 1408) f32r:
    #   [twT(2x128) pwT(2x128) gwT(2x128) wwT(256) ident(128) wraw(2x128)]
    wp_s = consts.tile([128, 1472], F32R)
    nc.sync.dma_start(out=wp_s, in_=wpack)
    tw_s = wp_s[:, 0:256].rearrange("p (k c) -> p k c", k=2)
    pw_s = wp_s[:, 256:512].rearrange("p (k c) -> p k c", k=2)
    gw_s = wp_s[:, 512:768].rearrange("p (k c) -> p k c", k=2)
    ww_s = wp_s[:, 768:1024]
    ident_s = wp_s[:, 1024:1152]
    wraw_s = wp_s[:, 1152:1408].rearrange("p (k c) -> p k c", k=2)
    # bf16 identity for the bf16 g transposes (packed as raw bits in wpack)
    identb_s = wp_s[:, 1408:1472].bitcast(BF16)
    # vpack (128, 5) f32: [tb, gamma(2), beta(2)]
    vp_s = consts.tile([128, 5], F32)
    nc.sync.dma_start(out=vp_s, in_=vpack)
    tb_s = vp_s[:, 0:1]
    gamma_s = vp_s[:, 1:3]
    beta_s = vp_s[:, 3:5]

    acc_s = consts.tile([128, 2 * BLOC * NMC], F32)   # per (cc, b, mc) wz sums
    zz_ps = pzz.tile([128, 128], F32)                 # z Gram matrix accumulator

    wz_tiles = {}
    n_zz = 0
    zz_last = 2 * BLOC * NMC * 2 - 1                  # 64 accumulated Gram matmuls

    x_tiles = {}
    for b in range(BLOC):
        # ---------------- load x (resident; also used by the residual) ------
        x_t = xfp.tile([128, 2, N], F32R, tag="xf", name=f"x_{b}")
        x_tiles[b] = x_t
        xs = x[b].rearrange("(k p) m -> p k m", p=128)
        for q in range(4):
            qs = slice(q * (N // 4), (q + 1) * (N // 4))
            nc.sync.dma_start(out=x_t[:, :, qs], in_=xs[:, :, qs])

        # ---------------- theta conv: (128ci, 4096) ----------------
        theta = thp.tile([128, N], F32R, tag="theta")
        for mc in range(NMC):
            ms = slice(mc * MC, (mc + 1) * MC)
            tps = pbank.tile([128, MC], F32, tag="bank")
            nc.tensor.matmul(tps, tw_s[:, 0, :], x_t[:, 0, ms], start=True, stop=False)
            nc.tensor.matmul(tps, tw_s[:, 1, :], x_t[:, 1, ms], start=False, stop=True)
            nc.scalar.activation(theta[:, ms], tps, ACT.Identity, bias=tb_s, scale=1.0)

        # ---------------- phi/g convs + 2x2 maxpool ----------------
        # pooled tensors: (128ci, 32ph, 32pw)
        phi_p = poolp.tile([128, 32, 32], F32R, tag="phi_p")
        g_p = poolp.tile([128, 32, 32], BF16, tag="g_p")
        ys = y[b].rearrange("(k p) m -> p k m", p=128)
        for mc in range(NMC):
            ms = slice(mc * MC, (mc + 1) * MC)
            if mc % 2 == 0:
                yr = yin.tile([128, 2, 2 * MC], F32R, tag="yin")
                nc.sync.dma_start(out=yr, in_=ys[:, :, mc * MC:(mc + 2) * MC])
            half = slice((mc % 2) * MC, (mc % 2 + 1) * MC)
            for which, w_s, dst in (("phi", pw_s, phi_p), ("g", gw_s, g_p)):
                cps = pbank.tile([128, MC], F32, tag="bank", name=f"cps_{which}")
                nc.tensor.matmul(cps, w_s[:, 0, :], yr[:, 0, half], start=True, stop=False)
                nc.tensor.matmul(cps, w_s[:, 1, :], yr[:, 1, half], start=False, stop=True)
                # 2x2 maxpool in one reduce: (128, 4ph, 32pw, 2hh, 2ww) -> XY
                v = cps.rearrange("p (ph hh pw ww) -> p ph pw hh ww", ph=4, hh=2, ww=2)
                nc.vector.tensor_reduce(
                    out=dst[:, mc * 4:(mc + 1) * 4, :], in_=v, axis=AX.XY, op=ALU.max,
                )

        # ---------------- gT tiles with [ones | zeros] pad columns ----------
        # gt: (128 pooled, 8 pchunk, 256) ; [:, :, 0:128]=g^T, col 128=1, rest 0
        gt = gtp.tile([128, 8, 132], BF16, tag="gt")
        nc.sync.dma_start(out=gt[:, :, 128:132], in_=gpad)
        g_flat = g_p.rearrange("p a b -> p (a b)")
        for half in range(2):
            gtps = pbank.tile([128, 4, 128], BF16, tag="bank", name="gtps")
            for j in range(4):
                pch = half * 4 + j
                nc.tensor.transpose(
                    gtps[:, j, :], g_flat[:, pch * 128:(pch + 1) * 128],
                    identb_s,
                )
            nc.vector.tensor_copy(out=gt[:, half * 4:(half + 1) * 4, 0:128], in_=gtps)

        # ---------------- attention per m-chunk ----------------
        zt = ztp.tile([128, N], F32R, tag="zt")
        phi_flat = phi_p.rearrange("p a b -> p (a b)")
        for mc in range(NMC):
            ms = slice(mc * MC, (mc + 1) * MC)
            # fT tiles: (128 pooled, 512 m) for each of 8 pooled chunks; exp on ACT
            fexp = fxp.tile([128, 8, MC], BF16, tag="fexp")
            for half in range(4):
                fps = psf.tile([128, 2, MC], F32, tag="f")
                for i in range(2):
                    pch = half * 2 + i
                    nc.tensor.matmul(
                        fps[:, i, :],
                        phi_flat[:, pch * 128:(pch + 1) * 128],
                        theta[:, ms],
                        start=True, stop=True,
                    )
                nc.scalar.activation(fexp[:, 2 * half:2 * half + 2, :], fps, ACT.Exp)

            # z matmuls: out (128 m, [z | s | junk]) accumulated over 8 pooled
            # chunks; two m-subtiles per PSUM bank tile
            tp = pbank.tile([128, 4, 128], F32, tag="bank", name="tp")
            for j2 in range(2):
                zb = pbank.tile([128, 512], F32, tag="bank", name="zb")
                for i in range(2):
                    sub = j2 * 2 + i
                    for pch in range(8):
                        nc.tensor.matmul(
                            zb[:, i * 256:i * 256 + 132],
                            fexp[:, pch, sub * 128:(sub + 1) * 128],
                            gt[:, pch, :],
                            start=(pch == 0), stop=(pch == 7),
                        )
                zb2 = zb.rearrange("p (i c) -> p i c", i=2)
                rc = ptmp.tile([128, 2], F32, tag="rc", bufs=4)
                nc.vector.reciprocal(rc, zb2[:, :, 128])
                zn2 = znp.tile([128, 2, 128], F32R, tag="zn")
                nc.vector.tensor_tensor(
                    zn2, zb2[:, :, 0:128],
                    rc[:, :, None].to_broadcast((128, 2, 128)), ALU.mult,
                )
                for i in range(2):
                    sub = j2 * 2 + i
                    nc.tensor.transpose(
                        tp[:, sub, :].bitcast(F32R), zn2[:, i, :], ident_s
                    )
                    # Gram accumulation for BN variance: ZZ += z_m^T z_m
                    nc.tensor.matmul(
                        zz_ps, zn2[:, i, :], zn2[:, i, :],
                        start=(n_zz == 0), stop=(n_zz == zz_last),
                        skip_group_check=True,
                    )
                    n_zz += 1
            nc.vector.tensor_copy(out=zt[:, ms], in_=tp.rearrange("p a b -> p (a b)"))

        # ---------------- W conv + BN partial sums ----------------
        for cc in range(2):
            wz_t = wzp.tile([128, N], BF16, tag="wz", name=f"wz_{b}_{cc}")
            wz_tiles[(b, cc)] = wz_t
            for mc in range(NMC):
                ms = slice(mc * MC, (mc + 1) * MC)
                idx = (cc * BLOC + b) * NMC + mc
                wb = pbank.tile([128, MC], F32, tag="bank", name="wb")
                nc.tensor.matmul(
                    wb, ww_s[:, cc * 128:(cc + 1) * 128], zt[:, ms],
                    start=True, stop=True,
                )
                nc.vector.tensor_scalar(
                    wz_t[:, ms], wb, 1.0, 0.0, ALU.mult, ALU.add,
                    accum_out=acc_s[:, idx:idx + 1],
                )

    # ---------------- global BN stats via AllReduce ----------------
    # local per-channel sums: ls[:, cc] = sum(wz), ls[:, 2+cc] = sum(wz^2)
    ls = consts.tile([128, 4], F32)
    for cc in range(2):
        cs = slice(cc * BLOC * NMC, (cc + 1) * BLOC * NMC)
        nc.vector.reduce_sum(out=ls[:, cc:cc + 1], in_=acc_s[:, cs], axis=AX.X)

    # sum(wz^2)_c = diag(W ZZ W^T): U = W_cc ZZ ; q_c = sum_cj U[c,cj] W[c,cj]
    zz_s = consts.tile([128, 128], F32R)
    nc.vector.tensor_copy(out=zz_s, in_=zz_ps)
    for cc in range(2):
        u_ps = pbank.tile([128, 128], F32, tag="bank", name="u_ps")
        nc.tensor.matmul(u_ps, ww_s[:, cc * 128:(cc + 1) * 128], zz_s,
                         start=True, stop=True)
        qjunk = ptmp.tile([128, 128], F32, tag="qjunk", bufs=1)
        nc.vector.scalar_tensor_tensor(
            qjunk, u_ps, 1.0, wraw_s[:, cc, :].bitcast(F32), ALU.mult, ALU.mult,
            accum_out=ls[:, 2 + cc:3 + cc],
        )

    cc_in = dram.tile([128, 4], F32)
    cc_out = dram.tile([128, 4], F32)
    nc.sync.dma_start(out=cc_in, in_=ls)
    if io.get("single_core_sim"):
        # stand-in for the AllReduce so TimelineSim (single-core) can run
        nc.sync.dma_start(out=cc_out, in_=cc_in)
    else:
        nc.gpsimd.collective_compute(
            "AllReduce", ALU.add,
            replica_groups=[list(range(NCORES))],
            ins=[cc_in.opt()], outs=[cc_out.opt()],
        )
    gs = consts.tile([128, 4], F32)
    nc.sync.dma_start(out=gs, in_=cc_out)

    inv = 1.0 / COUNT
    mean = consts.tile([128, 2], F32)
    nc.vector.tensor_scalar(mean, gs[:, 0:2], inv, None, ALU.mult)
    e2 = consts.tile([128, 2], F32)
    nc.vector.tensor_scalar(e2, gs[:, 2:4], inv, None, ALU.mult)
    msq = consts.tile([128, 2], F32)
    nc.vector.tensor_mul(msq, mean, mean)
    u = consts.tile([128, 2], F32)
    nc.vector.tensor_sub(u, e2, msq)
    nc.vector.tensor_scalar(u, u, EPS, None, ALU.add)
    # rsqrt(u) = exp(-0.5*ln(u)) -- Ln/Exp share the softmax's ACT table set
    y0 = consts.tile([128, 2], F32)
    nc.scalar.activation(y0, u, ACT.Ln)
    r0 = consts.tile([128, 2], F32)
    nc.scalar.activation(r0, y0, ACT.Exp, scale=-0.5)
    a_s = consts.tile([128, 2], F32)
    nc.vector.tensor_mul(a_s, r0, gamma_s)
    nb = consts.tile([128, 2], F32)
    nc.vector.tensor_mul(nb, mean, a_s)
    nc.vector.tensor_sub(nb, beta_s, nb)

    # ---------------- normalize + residual + store ----------------
    for b in range(BLOC):
        for cc in range(2):
            wz_t = wz_tiles[(b, cc)]
            csl = slice(cc * 128, (cc + 1) * 128)
            x_t = x_tiles[b]
            for half in range(2):
                hs = slice(half * 2048, (half + 1) * 2048)
                # wz <- wz*a + (beta - mean*a)  (in place, on ACT; chunked so
                # the residual adds and output DMAs pipeline behind it)
                nc.scalar.activation(
                    wz_t[:, hs], wz_t[:, hs], ACT.Identity,
                    bias=nb[:, cc:cc + 1], scale=a_s[:, cc:cc + 1],
                )
                for m2 in (2 * half, 2 * half + 1):
                    ms = slice(m2 * 1024, (m2 + 1) * 1024)
                    ot = outp.tile([128, 1024], F32, tag="ot")
                    eng = nc.vector if m2 % 2 == 0 else nc.gpsimd
                    eng.tensor_add(ot, wz_t[:, ms], x_t[:, cc, ms].bitcast(F32))
                    nc.sync.dma_start(out=out[b, csl, ms], in_=ot)


_CACHE = {}


def make_io(nc):
    return {
        "x": nc.dram_tensor("x", [BLOC, C, N], F32R, kind="ExternalInput").ap(),
        "y": nc.dram_tensor("y", [BLOC, C, N], F32R, kind="ExternalInput").ap(),
        "wpack": nc.dram_tensor("wpack", [128, 1472], F32R, kind="ExternalInput").ap(),
        "vpack": nc.dram_tensor("vpack", [128, 5], F32, kind="ExternalInput").ap(),
        "gpad": nc.dram_tensor("gpad", [128, 8, 4], BF16, kind="ExternalInput").ap(),
        "out": nc.dram_tensor("out", [BLOC, C, N], F32, kind="ExternalOutput").ap(),
    }


def _get_program():
    if "nc" in _CACHE:
        return _CACHE["nc"], _CACHE["io"]
    nc = bacc.Bacc(
        "TRN2", target_bir_lowering=False, debug=False,
        enable_asserts=False, num_devices=NCORES,
    )
    io = make_io(nc)
    from contextlib import ExitStack
    with tile.TileContext(nc) as tc:
        with ExitStack() as ctx:
            io["ctx"] = ctx
            build_body(tc, io)
    nc.compile()
    _CACHE["nc"] = nc
    _CACHE["io"] = io
    return nc, io


def kernel(x, y, theta_w, theta_b, phi_w, phi_b, g_w, g_b, W_w, W_b,
           bn_gamma, bn_beta, _trace=False, **_unused):
    x = np.asarray(x, dtype=np.float32).reshape(B, C, N)
    y = np.asarray(y, dtype=np.float32).reshape(B, C, N)

    def chunked(wT):
        # (C, CI) -> (128, 2, CI): [p, k, ci] = wT[k*128+p, ci]
        return np.asarray(wT, np.float32).reshape(2, 128, CI).transpose(1, 0, 2)

    tw = chunked(np.asarray(theta_w, np.float32).T)
    pw = chunked(np.asarray(phi_w, np.float32).T)
    gw = chunked(np.asarray(g_w, np.float32).T)
    ww = np.asarray(W_w, np.float32).T                             # (CI, C)
    wraw = chunked(np.asarray(W_w, np.float32))                    # c-part layout
    ident = np.eye(128, dtype=np.float32)
    # bf16 identity packed as raw bits into 64 f32 columns of wpack
    eye_bits = np.eye(128, dtype=np.float32).astype(ml_bf16).view(np.uint16).astype(np.uint32)
    packed = (eye_bits[:, 1::2] << 16) | eye_bits[:, 0::2]
    wpack = np.ascontiguousarray(np.concatenate([
        tw.reshape(128, 256), pw.reshape(128, 256), gw.reshape(128, 256),
        ww, ident, wraw.reshape(128, 256),
        packed.view(np.float32)], axis=1))
    tb = np.asarray(theta_b, np.float32).reshape(CI, 1)
    gamma = np.asarray(bn_gamma, np.float32).reshape(2, 128).T
    beta = np.asarray(bn_beta, np.float32).reshape(2, 128).T
    vpack = np.ascontiguousarray(np.concatenate([tb, gamma, beta], axis=1))
    gpad = np.zeros((128, 8, 4), ml_bf16)
    gpad[:, :, 0] = 1.0
    # phi_b, g_b, W_b intentionally unused: softmax-invariant / cancelled by BN.

    nc, _ = _get_program()
    in_maps = []
    for k in range(NCORES):
        in_maps.append({
            "x": np.ascontiguousarray(x[k * BLOC:(k + 1) * BLOC]),
            "y": np.ascontiguousarray(y[k * BLOC:(k + 1) * BLOC]),
            "wpack": wpack, "vpack": vpack, "gpad": gpad,
        })
    res = run_bass_kernel_spmd(nc, in_maps, core_ids=list(range(NCORES)), trace=_trace)
    out = np.concatenate([r_["out"] for r_ in res.results], axis=0)
    if _trace:
        _CACHE["last_results"] = res
    return out.reshape(B, C, 64, 64)



# revision 28
# speedup vs baseline: 1.3003x; 1.3003x over previous
# Trainium2 Bass kernel for the non-local attention block (nn_DRAL_88476326297980).
#
# Reference computation (per batch b):
#   theta = theta_w @ x_b + theta_b            (CI=128, N=4096)
#   phi   = maxpool2x2(phi_w @ y_b + phi_b)    (CI=128, P=1024)
#   g     = maxpool2x2(g_w  @ y_b + g_b)       (CI=128, P=1024)
#   f     = theta^T @ phi                      (N, P)
#   fdiv  = softmax(f, axis=P)
#   z     = fdiv @ g^T                         (N, CI)
#   wz    = W_w @ z^T + W_b                    (C=256, N)
#   out   = BN(wz over all b,n) + x            (training-mode batch stats)
#
# Sharding: data-parallel over batch, 2 batches per core, 8 cores.
# BN batch statistics are combined with a tiny (128x4) AllReduce.
#
# Math simplifications used (exact, not approximations):
#  - phi_b adds a per-row constant to f -> softmax-invariant -> dropped.
#  - g_b adds a per-CI constant to z (softmax weights sum to 1) -> shifts wz
#    per-channel -> cancelled by the BN mean subtraction -> dropped.
#  - W_b shifts wz per-channel -> cancelled by BN mean subtraction -> dropped.
#  - BN stats come from z-statistics, NOT from a pass over wz:
#      sum_n wz[c,n]   = W_c . (sum_n z_n)            (one tiny matmul)
#      sum_n wz[c,n]^2 = diag(W ZZ W^T)[c],  ZZ = sum_n z_n z_n^T (PSUM Gram)
#    so the AllReduce fires right after the last z tile, and the W conv +
#    normalize + residual + store run as one fused streaming tail.
#
# Schedule:
#  - attention is software-pipelined: the f-matmuls of step t+1 are emitted
#    before the z-matmuls of step t, so the PE works while ACT runs the exps.
#  - batch 1's convs (theta/phi/g + maxpool) are emitted as "hook" pieces
#    interleaved into batch 0's attention steps, hiding them in engine slack.
#  - DMA order y0, x0, y1, x1 (phi/g need full y before attention can start).
#  - the residual add is done IN PLACE into the resident x tile, and the
#    output DMA streams straight from it (no staging buffers).
#  - z path is bf16 (Gram/transposes at full PE rate); f path stays f32r.

import numpy as np
from ml_dtypes import bfloat16 as ml_bf16

import concourse.bass as bass
import concourse.mybir as mybir
import concourse.tile as tile
from concourse import bacc
from concourse.bass_utils import run_bass_kernel_spmd

F32 = mybir.dt.float32
F32R = mybir.dt.float32r
BF16 = mybir.dt.bfloat16
ALU = mybir.AluOpType
ACT = mybir.ActivationFunctionType
AX = mybir.AxisListType

NCORES = 8
B = 16
BLOC = B // NCORES          # 2 batches per core
C = 256                     # in channels
CI = 128                    # inter channels
N = 4096                    # h*w
MC = 512                    # m-chunk (columns per matmul)
NMC = N // MC               # 8
EPS = 1e-5
COUNT = B * N               # BN sample count per channel


def build_body(tc, io):
    nc = tc.nc
    x, y, wpack, vpack, gpad, out = (
        io["x"], io["y"], io["wpack"], io["vpack"], io["gpad"], io["out"],
    )

    ctx = io["ctx"]
    consts = ctx.enter_context(tc.tile_pool(name="consts", bufs=1))
    xfp = ctx.enter_context(tc.tile_pool(name="xfp", bufs=2))
    yin = ctx.enter_context(tc.tile_pool(name="yin", bufs=4))
    thp = ctx.enter_context(tc.tile_pool(name="thp", bufs=2))
    poolp = ctx.enter_context(tc.tile_pool(name="poolp", bufs=2))
    ptmp = ctx.enter_context(tc.tile_pool(name="ptmp", bufs=1))
    gtp = ctx.enter_context(tc.tile_pool(name="gtp", bufs=2))
    fxp = ctx.enter_context(tc.tile_pool(name="fxp", bufs=2))
    znp = ctx.enter_context(tc.tile_pool(name="znp", bufs=6))
    ztp = ctx.enter_context(tc.tile_pool(name="ztp", bufs=2))
    outp = ctx.enter_context(tc.tile_pool(name="outp", bufs=4))
    psf = ctx.enter_context(tc.tile_pool(name="psf", bufs=2, space="PSUM"))
    pbank = ctx.enter_context(tc.tile_pool(name="pbank", bufs=3, space="PSUM"))
    pzz = ctx.enter_context(tc.tile_pool(name="pzz", bufs=1, space="PSUM"))
    dram = ctx.enter_context(tc.tile_pool(name="dram", bufs=1, space="DRAM"))

    # ---- constants / weights: two packed DMAs to keep sync fan-in tiny ----
    # wpack (128, 1472) f32r:
    #   [twT(2x128) pwT(2x128) gwT(2x128) wwT(256) ident(128) wraw(2x128)
    #    identb_bits(64)]
    wp_s = consts.tile([128, 1472], F32R)
    nc.sync.dma_start(out=wp_s, in_=wpack)
    tw_s = wp_s[:, 0:256].rearrange("p (k c) -> p k c", k=2)
    pw_s = wp_s[:, 256:512].rearrange("p (k c) -> p k c", k=2)
    gw_s = wp_s[:, 512:768].rearrange("p (k c) -> p k c", k=2)
    ww_s = wp_s[:, 768:1024]
    wraw_s = wp_s[:, 1152:1408].rearrange("p (k c) -> p k c", k=2)
    # bf16 identity for PE transposes (packed as raw bits in wpack)
    identb_s = wp_s[:, 1408:1472].bitcast(BF16)
    # vpack (128, 5) f32: [tb, gamma(2), beta(2)]
    vp_s = consts.tile([128, 5], F32)
    nc.sync.dma_start(out=vp_s, in_=vpack)
    tb_s = vp_s[:, 0:1]
    gamma_s = vp_s[:, 1:3]
    beta_s = vp_s[:, 3:5]

    zsacc = consts.tile([128, BLOC * NMC], F32)       # per (b, mc) z row-sums
    zz_ps = pzz.tile([128, 128], F32)                 # z Gram matrix accumulator

    # bf16 copy of W^T for matmuls whose rhs is bf16 (the hardware requires
    # matching dtypes when either operand is f32/f32r)
    wwb_s = consts.tile([128, 256], BF16)
    nc.vector.tensor_copy(out=wwb_s, in_=ww_s)

    n_zz = 0
    zz_last = BLOC * NMC * 4 - 1                      # 64 accumulated Gram matmuls

    # ---------------- input DMAs: gpads, then y0, x0, y1, x1 ----------------
    x_tiles, y_views, x_views = {}, {}, {}
    yr_tiles = {}
    gt_t = {}
    for b in range(BLOC):
        x_t = xfp.tile([128, 2, N], F32R, tag="xf", name=f"x_{b}")
        x_tiles[b] = x_t
        x_views[b] = x[b].rearrange("(k p) m -> p k m", p=128)
        y_views[b] = y[b].rearrange("(k p) m -> p k m", p=128)
        # gT tiles with [ones | zeros] pad columns: (128, 8, 132) bf16.
        # Tiny pad DMA issued before the big input streams.
        gt_t[b] = gtp.tile([128, 8, 132], BF16, tag="gt", name=f"gt_{b}")
        nc.sync.dma_start(out=gt_t[b][:, :, 128:132], in_=gpad)

    def issue_y(b, c):
        yr = yin.tile([128, 2, MC], F32R, tag="yin", name=f"y_{b}_{c}")
        yr_tiles[(b, c)] = yr
        nc.sync.dma_start(out=yr, in_=y_views[b][:, :, c * MC:(c + 1) * MC])

    def issue_x(b, q):
        qs = slice(q * MC, (q + 1) * MC)
        nc.sync.dma_start(out=x_tiles[b][:, :, qs], in_=x_views[b][:, :, qs])

    # DMAs are served FIFO at full aggregate bandwidth, so issue order is
    # arrival order: y0 first (attention needs the FULL pooled phi/g), with
    # x0 q0/q1 slotted early for the first theta pieces.
    issue_y(0, 0)
    issue_y(0, 1)
    issue_x(0, 0)
    for c in range(2, NMC):
        issue_y(0, c)
    for q in range(1, NMC):
        issue_x(0, q)
    for c in range(NMC):
        issue_y(1, c)
    for q in range(NMC):
        issue_x(1, q)

    def small_dma(out_, in_):
        # stats-path DMAs ride the Pool queue: 25ns issue vs 565ns on sync
        nc.gpsimd.dma_start(out=out_, in_=in_)

    # ---------------- per-batch state ----------------
    theta_t, phi_t, g_t = {}, {}, {}
    zt_t = {}
    for b in range(BLOC):
        theta_t[b] = thp.tile([128, N], F32R, tag="theta", name=f"theta_{b}")
        phi_t[b] = poolp.tile([128, 32, 32], F32R, tag="phi_p", name=f"phi_{b}")
        g_t[b] = poolp.tile([128, 32, 32], BF16, tag="g_p", name=f"g_{b}")
        zt_t[b] = ztp.tile([128, N], BF16, tag="zt", name=f"zt_{b}")

    def conv_piece(b, mc):
        """phi/g convs + 2x2 maxpool for y columns [mc*512, (mc+1)*512)."""
        yr = yr_tiles[(b, mc)]
        for which, w_s, dst in (("phi", pw_s, phi_t[b]), ("g", gw_s, g_t[b])):
            cps = pbank.tile([128, MC], F32, tag="bank", name=f"cps_{which}")
            nc.tensor.matmul(cps, w_s[:, 0, :], yr[:, 0, :], start=True, stop=False)
            nc.tensor.matmul(cps, w_s[:, 1, :], yr[:, 1, :], start=False, stop=True)
            # 2x2 maxpool in one reduce: (128, 4ph, 32pw, 2hh, 2ww) -> XY
            v = cps.rearrange("p (ph hh pw ww) -> p ph pw hh ww", ph=4, hh=2, ww=2)
            nc.vector.tensor_reduce(
                out=dst[:, mc * 4:(mc + 1) * 4, :], in_=v, axis=AX.XY, op=ALU.max,
            )

    def theta_piece(b, mc, on_act=False):
        """theta conv for x columns [mc*512 ...): PE matmuls + copy w/ bias.

        The PSUM->SBUF copy goes to DVE by default (ACT is the binding
        engine in the attention steps); on_act=True for steps where the
        maxpool reduces already load DVE.
        """
        ms = slice(mc * MC, (mc + 1) * MC)
        tps = pbank.tile([128, MC], F32, tag="bank", name="tps")
        nc.tensor.matmul(tps, tw_s[:, 0, :], x_tiles[b][:, 0, ms], start=True, stop=False)
        nc.tensor.matmul(tps, tw_s[:, 1, :], x_tiles[b][:, 1, ms], start=False, stop=True)
        if on_act:
            nc.scalar.activation(theta_t[b][:, ms], tps, ACT.Identity,
                                 bias=tb_s, scale=1.0)
        else:
            # tensor_tensor with a broadcast bias: DVE op form that carries
            # the round-to-f32r flag the BIR verifier requires of producers
            # feeding f32r matmuls (tensor_scalar does not)
            nc.vector.tensor_tensor(
                theta_t[b][:, ms].rearrange("p (a m) -> p a m", a=1),
                tps.rearrange("p (a m) -> p a m", a=1),
                tb_s[:, :, None].to_broadcast((128, 1, MC)),
                ALU.add,
            )

    def build_gt(b):
        """fill gT tile columns 0:128 by PE-transposing the pooled g."""
        gt = gt_t[b]
        g_flat = g_t[b].rearrange("p a b -> p (a b)")
        for half in range(2):
            gtps = pbank.tile([128, 4, 128], BF16, tag="bank", name="gtps")
            for j in range(4):
                pch = half * 4 + j
                nc.tensor.transpose(
                    gtps[:, j, :], g_flat[:, pch * 128:(pch + 1) * 128], identb_s,
                )
            nc.vector.tensor_copy(out=gt[:, half * 4:(half + 1) * 4, 0:128], in_=gtps)

    # ---------------- attention steps, software-pipelined ----------------
    fexp_t = {}

    def emit_f(b, mc):
        """f matmuls (fT layout) + exp on ACT -> fexp (128, 8, 512) bf16."""
        ms = slice(mc * MC, (mc + 1) * MC)
        fexp = fxp.tile([128, 8, MC], BF16, tag="fexp", name=f"fexp_{b}_{mc}")
        fexp_t[(b, mc)] = fexp
        phi_flat = phi_t[b].rearrange("p a b -> p (a b)")
        for half in range(4):
            fps = psf.tile([128, 2, MC], F32, tag="f")
            for i in range(2):
                pch = half * 2 + i
                nc.tensor.matmul(
                    fps[:, i, :],
                    phi_flat[:, pch * 128:(pch + 1) * 128],
                    theta_t[b][:, ms],
                    start=True, stop=True,
                )
            nc.scalar.activation(fexp[:, 2 * half:2 * half + 2, :], fps, ACT.Exp)

    def emit_z(b, mc):
        """z matmuls + normalize + transpose to zT (bf16) + Gram + row-sums.

        Both z matmul groups are emitted before the transposes/Gram so the
        PE never sits behind a transpose that waits on the DVE normalize.
        """
        nonlocal n_zz
        ms = slice(mc * MC, (mc + 1) * MC)
        fexp = fexp_t.pop((b, mc))
        gt = gt_t[b]
        tp = pbank.tile([128, 4, 128], BF16, tag="bank", name="tp")
        zbs = []
        for j2 in range(2):
            zb = pbank.tile([128, 512], F32, tag="bank", name="zb")
            zbs.append(zb)
            for i in range(2):
                sub = j2 * 2 + i
                for pch in range(8):
                    nc.tensor.matmul(
                        zb[:, i * 256:i * 256 + 132],
                        fexp[:, pch, sub * 128:(sub + 1) * 128],
                        gt[:, pch, :],
                        start=(pch == 0), stop=(pch == 7),
                    )
        zns = []
        for zb in zbs:
            zb2 = zb.rearrange("p (i c) -> p i c", i=2)
            rc = ptmp.tile([128, 2], F32, tag="rc", bufs=4)
            nc.vector.reciprocal(rc, zb2[:, :, 128])
            zn2 = znp.tile([128, 2, 128], BF16, tag="zn")
            nc.vector.tensor_tensor(
                zn2, zb2[:, :, 0:128],
                rc[:, :, None].to_broadcast((128, 2, 128)), ALU.mult,
            )
            zns.append(zn2)
        # transposes back-to-back (shared identity Ldweights), then Gram
        for j2, zn2 in enumerate(zns):
            for i in range(2):
                nc.tensor.transpose(tp[:, j2 * 2 + i, :], zn2[:, i, :], identb_s)
        for zn2 in zns:
            for i in range(2):
                # Gram accumulation for BN variance: ZZ += z_m^T z_m
                nc.tensor.matmul(
                    zz_ps, zn2[:, i, :], zn2[:, i, :],
                    start=(n_zz == 0), stop=(n_zz == zz_last),
                    skip_group_check=True,
                )
                n_zz += 1
        # PSUM -> SBUF zT copy; accum_out gives per-ci row sums over this mc
        idx = b * NMC + mc
        nc.vector.tensor_scalar(
            zt_t[b][:, ms], tp.rearrange("p a b -> p (a b)"), 1.0, 0.0,
            ALU.mult, ALU.add, accum_out=zsacc[:, idx:idx + 1],
        )

    # hook work interleaved into batch-0's attention steps
    hooks = {}
    for j in range(6):                       # b0 theta pieces 2..7
        hooks.setdefault((0, j), []).append(
            lambda b=0, mc=j + 2, oa=(2 <= j <= 5): theta_piece(b, mc, on_act=oa))
    for c in range(8):                       # b1 convs at steps 2..5
        hooks.setdefault((0, c // 2 + 2), []).append(lambda c=c: conv_piece(1, c))
    hooks.setdefault((0, 6), []).append(lambda: build_gt(1))
    hooks.setdefault((0, 6), []).append(lambda: theta_piece(1, 0))
    hooks.setdefault((0, 6), []).append(lambda: theta_piece(1, 1))
    hooks.setdefault((0, 7), []).append(lambda: theta_piece(1, 2))
    hooks.setdefault((0, 7), []).append(lambda: theta_piece(1, 3))
    for j in range(4):                       # b1 theta pieces 4..7
        hooks.setdefault((1, j), []).append(lambda mc=j + 4: theta_piece(1, mc))

    # PE p-state warm-up: the PE ramps to full clock only after ~4.5us of
    # continuous execution (one-time). Burn the ramp on junk matmuls over the
    # already-resident weight pack while the input DMAs stream.
    for w in range(6):
        wu = psf.tile([128, 2, MC], F32, tag="f", name="warm")
        nc.tensor.matmul(wu[:, 0, :], tw_s[:, 0, :], wp_s[:, 0:MC],
                         start=True, stop=True, skip_group_check=True)
        nc.tensor.matmul(wu[:, 1, :], tw_s[:, 0, :], wp_s[:, MC:2 * MC],
                         start=True, stop=True, skip_group_check=True)

    # batch-0 convs (DMA-gated startup) + first theta pieces; gt build last
    # (z(0,0) needs it ~4us after f(0,0), theta/phi gate f(0,0) itself)
    conv_piece(0, 0)
    conv_piece(0, 1)
    theta_piece(0, 0)
    for mc in range(2, NMC):
        conv_piece(0, mc)
    theta_piece(0, 1)
    build_gt(0)

    steps = [(0, mc) for mc in range(NMC)] + [(1, mc) for mc in range(NMC)]
    emit_f(*steps[0])
    for t, (b, mc) in enumerate(steps):
        if t + 1 < len(steps):
            emit_f(*steps[t + 1])
        for fn in hooks.get((b, mc), ()):
            fn()
        emit_z(b, mc)

    # ---------------- BN stats from z-statistics + AllReduce ----------------
    # ls[:, cc] = sum_n wz = W_cc . zsum ; ls[:, 2+cc] = sum_n wz^2 (Gram)
    ls = consts.tile([128, 4], F32)
    # var = E[wz^2] - mean^2 cancels catastrophically here (z columns share a
    # large common mean: mean^2 is up to 99% of E[wz^2], a ~170x error
    # amplifier), so the stats path needs two properties:
    #  - CONSISTENCY: measure moments of exactly the wz the tail computes,
    #    i.e. every W factor is the same bf16-rounded W (wwb/wrawb).
    #  - PRECISION: ZZ and zsum enter bf16 matmuls as hi/lo bf16 splits
    #    (two accumulated matmuls ~= a 16-bit mantissa, rel err ~2e-5).
    wrawb = consts.tile([128, 2, 128], BF16)
    nc.vector.tensor_copy(out=wrawb, in_=wraw_s)
    zz_hi = consts.tile([128, 128], BF16)
    nc.vector.tensor_copy(out=zz_hi, in_=zz_ps)
    zz_lo = consts.tile([128, 128], BF16)
    nc.vector.tensor_sub(zz_lo, zz_ps, zz_hi)
    zsum = consts.tile([128, 1], F32)
    nc.vector.reduce_sum(out=zsum, in_=zsacc, axis=AX.X)
    zs_hi = consts.tile([128, 1], BF16)
    nc.vector.tensor_copy(out=zs_hi, in_=zsum)
    zs_lo = consts.tile([128, 1], BF16)
    nc.vector.tensor_sub(zs_lo, zsum, zs_hi)
    mps = pbank.tile([128, 2], F32, tag="bank", name="mps")
    for cc in range(2):
        wc = wwb_s[:, cc * 128:(cc + 1) * 128]
        nc.tensor.matmul(mps[:, cc:cc + 1], wc, zs_hi, start=True, stop=False,
                         skip_group_check=True)
        nc.tensor.matmul(mps[:, cc:cc + 1], wc, zs_lo, start=False, stop=True,
                         skip_group_check=True)
        u_ps = pbank.tile([128, 128], F32, tag="bank", name="u_ps")
        nc.tensor.matmul(u_ps, wc, zz_hi, start=True, stop=False)
        nc.tensor.matmul(u_ps, wc, zz_lo, start=False, stop=True)
        qjunk = ptmp.tile([128, 128], F32, tag="qjunk", bufs=1)
        nc.vector.scalar_tensor_tensor(
            qjunk, u_ps, 1.0, wrawb[:, cc, :], ALU.mult, ALU.mult,
            accum_out=ls[:, 2 + cc:3 + cc],
        )
    nc.vector.tensor_copy(out=ls[:, 0:2], in_=mps)

    cc_in = dram.tile([128, 4], F32)
    cc_out = dram.tile([128, 4], F32)
    small_dma(cc_in, ls)
    if io.get("single_core_sim"):
        # stand-in for the AllReduce so TimelineSim (single-core) can run
        small_dma(cc_out, cc_in)
    else:
        nc.gpsimd.collective_compute(
            "AllReduce", ALU.add,
            replica_groups=[list(range(NCORES))],
            ins=[cc_in.opt()], outs=[cc_out.opt()],
        )
    gs = consts.tile([128, 4], F32)
    small_dma(gs, cc_out)

    inv = 1.0 / COUNT
    me2 = consts.tile([128, 4], F32)          # [mean(2) | E[wz^2](2)]
    nc.vector.tensor_scalar(me2, gs, inv, None, ALU.mult)
    mean = me2[:, 0:2]
    u = consts.tile([128, 2], F32)
    nc.vector.tensor_tensor(u, mean, mean, ALU.mult)
    nc.vector.tensor_sub(u, me2[:, 2:4], u)
    nc.vector.tensor_scalar(u, u, EPS, None, ALU.add)
    # table-free rsqrt on DVE (quake seed + 2 Newton steps). The Ln/Exp pair
    # used before forced two LoadActFuncSet round-trips (~2.6us) onto the
    # post-AllReduce critical path; this chain never touches the ACT tables.
    I32 = mybir.dt.int32
    USE_QUAKE = False
    if USE_QUAKE:
        t1 = consts.tile([128, 2], I32)
        nc.vector.tensor_scalar(t1, u.bitcast(I32), 1, None, ALU.logical_shift_right)
        t2 = consts.tile([128, 2], I32)
        nc.vector.tensor_scalar(t2, t1, -1, 0x5F3759DF, ALU.mult, ALU.add)
        r0 = t2.bitcast(F32)
    else:
        y0 = consts.tile([128, 2], F32)
        nc.scalar.activation(y0, u, ACT.Ln)
        r0 = consts.tile([128, 2], F32)
        nc.scalar.activation(r0, y0, ACT.Exp, scale=-0.5)
    for it in range(2):
        uy2 = consts.tile([128, 2], F32, name=f"uy2_{it}")
        nc.vector.tensor_mul(uy2, r0, r0)
        nc.vector.tensor_mul(uy2, uy2, u)
        half3 = consts.tile([128, 2], F32, name=f"half3_{it}")
        nc.vector.tensor_scalar(half3, uy2, -0.5, 1.5, ALU.mult, ALU.add)
        r1 = consts.tile([128, 2], F32, name=f"rs_{it}")
        nc.vector.tensor_mul(r1, r0, half3)
        r0 = r1
    a_s = consts.tile([128, 2], F32)
    nc.vector.tensor_mul(a_s, r0, gamma_s)
    nb = consts.tile([128, 2], F32)
    nc.vector.tensor_mul(nb, mean, a_s)
    nc.vector.tensor_sub(nb, beta_s, nb)

    # ---------------- fused tail: W conv + normalize + residual + store ----
    for b in range(BLOC):
        for cc in range(2):
            csl = slice(cc * 128, (cc + 1) * 128)
            for mc in range(NMC):
                ms = slice(mc * MC, (mc + 1) * MC)
                k = (b * 2 + cc) * NMC + mc
                wpool, wtag = (pbank, "bank") if k % 2 == 0 else (psf, "f")
                wb = wpool.tile([128, MC], F32, tag=wtag, name="wb")
                nc.tensor.matmul(wb, wwb_s[:, cc * 128:(cc + 1) * 128],
                                 zt_t[b][:, ms], start=True, stop=True)
                wn = outp.tile([128, MC], F32, tag="wn")
                nc.scalar.activation(wn, wb, ACT.Identity,
                                     bias=nb[:, cc:cc + 1], scale=a_s[:, cc:cc + 1])
                # residual in place into the resident x tile, DMA straight
                # out. The add writes through the F32R view: x_t is also read
                # by the (long done) f32r theta matmuls, and the BIR verifier
                # statically requires every writer of an f32r-matmul input to
                # round. Costs ~2.4e-4 relative rounding on the output.
                xr = x_tiles[b][:, cc, ms]
                xv = xr.bitcast(F32)
                if k % 3 == 2:
                    nc.gpsimd.tensor_add(xr, xv, wn)
                else:
                    nc.vector.tensor_add(xr, xv, wn)
                nc.sync.dma_start(out=out[b, csl, ms], in_=xv)


_CACHE = {}


def make_io(nc):
    return {
        "x": nc.dram_tensor("x", [BLOC, C, N], F32R, kind="ExternalInput").ap(),
        "y": nc.dram_tensor("y", [BLOC, C, N], F32R, kind="ExternalInput").ap(),
        "wpack": nc.dram_tensor("wpack", [128, 1472], F32R, kind="ExternalInput").ap(),
        "vpack": nc.dram_tensor("vpack", [128, 5], F32, kind="ExternalInput").ap(),
        "gpad": nc.dram_tensor("gpad", [128, 8, 4], BF16, kind="ExternalInput").ap(),
        "out": nc.dram_tensor("out", [BLOC, C, N], F32, kind="ExternalOutput").ap(),
    }


def _get_program():
    if "nc" in _CACHE:
        return _CACHE["nc"], _CACHE["io"]
    nc = bacc.Bacc(
        "TRN2", target_bir_lowering=False, debug=False,
        enable_asserts=False, num_devices=NCORES,
    )
    io = make_io(nc)
    from contextlib import ExitStack
    with tile.TileContext(nc) as tc:
        with ExitStack() as ctx:
            io["ctx"] = ctx
            build_body(tc, io)
    nc.compile()
    _CACHE["nc"] = nc
    _CACHE["io"] = io
    return nc, io


def kernel(x, y, theta_w, theta_b, phi_w, phi_b, g_w, g_b, W_w, W_b,
           bn_gamma, bn_beta, _trace=False, **_unused):
    x = np.asarray(x, dtype=np.float32).reshape(B, C, N)
    y = np.asarray(y, dtype=np.float32).reshape(B, C, N)

    def chunked(wT):
        # (C, CI) -> (128, 2, CI): [p, k, ci] = wT[k*128+p, ci]
        return np.asarray(wT, np.float32).reshape(2, 128, CI).transpose(1, 0, 2)

    tw = chunked(np.asarray(theta_w, np.float32).T)
    pw = chunked(np.asarray(phi_w, np.float32).T)
    gw = chunked(np.asarray(g_w, np.float32).T)
    ww = np.asarray(W_w, np.float32).T                             # (CI, C)
    wraw = chunked(np.asarray(W_w, np.float32))                    # c-part layout
    ident = np.eye(128, dtype=np.float32)
    # bf16 identity packed as raw bits into 64 f32 columns of wpack
    eye_bits = np.eye(128, dtype=np.float32).astype(ml_bf16).view(np.uint16).astype(np.uint32)
    packed = (eye_bits[:, 1::2] << 16) | eye_bits[:, 0::2]
    wpack = np.ascontiguousarray(np.concatenate([
        tw.reshape(128, 256), pw.reshape(128, 256), gw.reshape(128, 256),
        ww, ident, wraw.reshape(128, 256),
        packed.view(np.float32)], axis=1))
    tb = np.asarray(theta_b, np.float32).reshape(CI, 1)
    gamma = np.asarray(bn_gamma, np.float32).reshape(2, 128).T
    beta = np.asarray(bn_beta, np.float32).reshape(2, 128).T
    vpack = np.ascontiguousarray(np.concatenate([tb, gamma, beta], axis=1))
    gpad = np.zeros((128, 8, 4), ml_bf16)
    gpad[:, :, 0] = 1.0
    # phi_b, g_b, W_b intentionally unused: softmax-invariant / cancelled by BN.

    nc, _ = _get_program()
    in_maps = []
    for k in range(NCORES):
        in_maps.append({
            "x": np.ascontiguousarray(x[k * BLOC:(k + 1) * BLOC]),
            "y": np.ascontiguousarray(y[k * BLOC:(k + 1) * BLOC]),
            "wpack": wpack, "vpack": vpack, "gpad": gpad,
        })
    res = run_bass_kernel_spmd(nc, in_maps, core_ids=list(range(NCORES)), trace=_trace)
    out = np.concatenate([r_["out"] for r_ in res.results], axis=0)
    if _trace:
        _CACHE["last_results"] = res
    return out.reshape(B, C, 64, 64)


# revision 29
# speedup vs baseline: 1.3142x; 1.0106x over previous
# Trainium2 Bass kernel for the non-local attention block (nn_DRAL_88476326297980).
#
# Reference computation (per batch b):
#   theta = theta_w @ x_b + theta_b            (CI=128, N=4096)
#   phi   = maxpool2x2(phi_w @ y_b + phi_b)    (CI=128, P=1024)
#   g     = maxpool2x2(g_w  @ y_b + g_b)       (CI=128, P=1024)
#   f     = theta^T @ phi                      (N, P)
#   fdiv  = softmax(f, axis=P)
#   z     = fdiv @ g^T                         (N, CI)
#   wz    = W_w @ z^T + W_b                    (C=256, N)
#   out   = BN(wz over all b,n) + x            (training-mode batch stats)
#
# Sharding: data-parallel over batch, 2 batches per core, 8 cores.
# BN batch statistics are combined with a tiny (128x4) AllReduce.
#
# Math simplifications used (exact, not approximations):
#  - phi_b adds a per-row constant to f -> softmax-invariant -> dropped.
#  - g_b adds a per-CI constant to z (softmax weights sum to 1) -> shifts wz
#    per-channel -> cancelled by the BN mean subtraction -> dropped.
#  - W_b shifts wz per-channel -> cancelled by BN mean subtraction -> dropped.
#  - BN stats come from z-statistics, NOT from a pass over wz:
#      sum_n wz[c,n]   = W_c . (sum_n z_n)            (one tiny matmul)
#      sum_n wz[c,n]^2 = diag(W ZZ W^T)[c],  ZZ = sum_n z_n z_n^T (PSUM Gram)
#    so the AllReduce fires right after the last z tile, and the W conv +
#    normalize + residual + store run as one fused streaming tail.
#
# Schedule:
#  - attention is software-pipelined: the f-matmuls of step t+1 are emitted
#    before the z-matmuls of step t, so the PE works while ACT runs the exps.
#  - batch 1's convs (theta/phi/g + maxpool) are emitted as "hook" pieces
#    interleaved into batch 0's attention steps, hiding them in engine slack.
#  - DMA order y0, x0, y1, x1 (phi/g need full y before attention can start).
#  - the residual add is done IN PLACE into the resident x tile, and the
#    output DMA streams straight from it (no staging buffers).
#  - z path is bf16 (Gram/transposes at full PE rate); f path stays f32r.

import numpy as np
from ml_dtypes import bfloat16 as ml_bf16

import concourse.bass as bass
import concourse.mybir as mybir
import concourse.tile as tile
from concourse import bacc
from concourse.bass_utils import run_bass_kernel_spmd

F32 = mybir.dt.float32
F32R = mybir.dt.float32r
BF16 = mybir.dt.bfloat16
ALU = mybir.AluOpType
ACT = mybir.ActivationFunctionType
AX = mybir.AxisListType

NCORES = 8
B = 16
BLOC = B // NCORES          # 2 batches per core
C = 256                     # in channels
CI = 128                    # inter channels
N = 4096                    # h*w
MC = 512                    # m-chunk (columns per matmul)
NMC = N // MC               # 8
EPS = 1e-5
COUNT = B * N               # BN sample count per channel


def build_body(tc, io):
    nc = tc.nc
    x, y, wpack, vpack, gpad, out = (
        io["x"], io["y"], io["wpack"], io["vpack"], io["gpad"], io["out"],
    )

    ctx = io["ctx"]
    consts = ctx.enter_context(tc.tile_pool(name="consts", bufs=1))
    xfp = ctx.enter_context(tc.tile_pool(name="xfp", bufs=2))
    yin = ctx.enter_context(tc.tile_pool(name="yin", bufs=4))
    thp = ctx.enter_context(tc.tile_pool(name="thp", bufs=2))
    poolp = ctx.enter_context(tc.tile_pool(name="poolp", bufs=2))
    ptmp = ctx.enter_context(tc.tile_pool(name="ptmp", bufs=1))
    gtp = ctx.enter_context(tc.tile_pool(name="gtp", bufs=2))
    fxp = ctx.enter_context(tc.tile_pool(name="fxp", bufs=2))
    znp = ctx.enter_context(tc.tile_pool(name="znp", bufs=6))
    ztp = ctx.enter_context(tc.tile_pool(name="ztp", bufs=2))
    outp = ctx.enter_context(tc.tile_pool(name="outp", bufs=4))
    psf = ctx.enter_context(tc.tile_pool(name="psf", bufs=2, space="PSUM"))
    pbank = ctx.enter_context(tc.tile_pool(name="pbank", bufs=3, space="PSUM"))
    pzz = ctx.enter_context(tc.tile_pool(name="pzz", bufs=1, space="PSUM"))
    dram = ctx.enter_context(tc.tile_pool(name="dram", bufs=1, space="DRAM"))

    # ---- constants / weights: two packed DMAs to keep sync fan-in tiny ----
    # wpack (128, 1472) f32r:
    #   [twT(2x128) pwT(2x128) gwT(2x128) wwT(256) ident(128) wraw(2x128)
    #    identb_bits(64)]
    wp_s = consts.tile([128, 1472], F32R)
    nc.sync.dma_start(out=wp_s, in_=wpack)
    tw_s = wp_s[:, 0:256].rearrange("p (k c) -> p k c", k=2)
    pw_s = wp_s[:, 256:512].rearrange("p (k c) -> p k c", k=2)
    gw_s = wp_s[:, 512:768].rearrange("p (k c) -> p k c", k=2)
    ww_s = wp_s[:, 768:1024]
    wraw_s = wp_s[:, 1152:1408].rearrange("p (k c) -> p k c", k=2)
    # bf16 identity for PE transposes (packed as raw bits in wpack)
    identb_s = wp_s[:, 1408:1472].bitcast(BF16)
    # vpack (128, 5) f32: [tb, gamma(2), beta(2)]
    vp_s = consts.tile([128, 5], F32)
    nc.sync.dma_start(out=vp_s, in_=vpack)
    tb_s = vp_s[:, 0:1]
    gamma_s = vp_s[:, 1:3]
    beta_s = vp_s[:, 3:5]

    zsacc = consts.tile([128, BLOC * NMC], F32)       # per (b, mc) z row-sums
    zz_ps = pzz.tile([128, 128], F32)                 # z Gram matrix accumulator

    # bf16 copy of W^T for matmuls whose rhs is bf16 (the hardware requires
    # matching dtypes when either operand is f32/f32r)
    wwb_s = consts.tile([128, 256], BF16)
    nc.vector.tensor_copy(out=wwb_s, in_=ww_s)

    n_zz = 0
    zz_last = BLOC * NMC * 4 - 1                      # 64 accumulated Gram matmuls

    # ---------------- input DMAs: gpads, then y0, x0, y1, x1 ----------------
    x_tiles, y_views, x_views = {}, {}, {}
    yr_tiles = {}
    gt_t = {}
    for b in range(BLOC):
        x_t = xfp.tile([128, 2, N], F32R, tag="xf", name=f"x_{b}")
        x_tiles[b] = x_t
        x_views[b] = x[b].rearrange("(k p) m -> p k m", p=128)
        y_views[b] = y[b].rearrange("(k p) m -> p k m", p=128)
        # gT tiles with [ones | zeros] pad columns: (128, 8, 132) bf16.
        # Tiny pad DMA issued before the big input streams.
        gt_t[b] = gtp.tile([128, 8, 132], BF16, tag="gt", name=f"gt_{b}")
        nc.sync.dma_start(out=gt_t[b][:, :, 128:132], in_=gpad)

    def issue_y(b, c):
        yr = yin.tile([128, 2, MC], F32R, tag="yin", name=f"y_{b}_{c}")
        yr_tiles[(b, c)] = yr
        nc.sync.dma_start(out=yr, in_=y_views[b][:, :, c * MC:(c + 1) * MC])

    def issue_x(b, q):
        qs = slice(q * MC, (q + 1) * MC)
        nc.sync.dma_start(out=x_tiles[b][:, :, qs], in_=x_views[b][:, :, qs])

    # DMAs are served FIFO at full aggregate bandwidth, so issue order is
    # arrival order: y0 first (attention needs the FULL pooled phi/g), with
    # x0 q0/q1 slotted early for the first theta pieces.
    issue_y(0, 0)
    issue_y(0, 1)
    issue_x(0, 0)
    for c in range(2, NMC):
        issue_y(0, c)
    for q in range(1, NMC):
        issue_x(0, q)
    for c in range(NMC):
        issue_y(1, c)
    for q in range(NMC):
        issue_x(1, q)

    def small_dma(out_, in_):
        # stats-path DMAs ride the Pool queue: 25ns issue vs 565ns on sync
        nc.gpsimd.dma_start(out=out_, in_=in_)

    # ---------------- per-batch state ----------------
    theta_t, phi_t, g_t = {}, {}, {}
    zt_t = {}
    for b in range(BLOC):
        theta_t[b] = thp.tile([128, N], F32R, tag="theta", name=f"theta_{b}")
        phi_t[b] = poolp.tile([128, 32, 32], F32R, tag="phi_p", name=f"phi_{b}")
        g_t[b] = poolp.tile([128, 32, 32], BF16, tag="g_p", name=f"g_{b}")
        zt_t[b] = ztp.tile([128, N], BF16, tag="zt", name=f"zt_{b}")

    def conv_piece(b, mc):
        """phi/g convs + 2x2 maxpool for y columns [mc*512, (mc+1)*512)."""
        yr = yr_tiles[(b, mc)]
        for which, w_s, dst in (("phi", pw_s, phi_t[b]), ("g", gw_s, g_t[b])):
            cps = pbank.tile([128, MC], F32, tag="bank", name=f"cps_{which}")
            nc.tensor.matmul(cps, w_s[:, 0, :], yr[:, 0, :], start=True, stop=False)
            nc.tensor.matmul(cps, w_s[:, 1, :], yr[:, 1, :], start=False, stop=True)
            # 2x2 maxpool in one reduce: (128, 4ph, 32pw, 2hh, 2ww) -> XY
            v = cps.rearrange("p (ph hh pw ww) -> p ph pw hh ww", ph=4, hh=2, ww=2)
            nc.vector.tensor_reduce(
                out=dst[:, mc * 4:(mc + 1) * 4, :], in_=v, axis=AX.XY, op=ALU.max,
            )

    def theta_piece(b, mc, on_act=False):
        """theta conv for x columns [mc*512 ...): PE matmuls + copy w/ bias.

        The PSUM->SBUF copy goes to DVE by default (ACT is the binding
        engine in the attention steps); on_act=True for steps where the
        maxpool reduces already load DVE.
        """
        ms = slice(mc * MC, (mc + 1) * MC)
        tps = pbank.tile([128, MC], F32, tag="bank", name="tps")
        nc.tensor.matmul(tps, tw_s[:, 0, :], x_tiles[b][:, 0, ms], start=True, stop=False)
        nc.tensor.matmul(tps, tw_s[:, 1, :], x_tiles[b][:, 1, ms], start=False, stop=True)
        if on_act:
            nc.scalar.activation(theta_t[b][:, ms], tps, ACT.Identity,
                                 bias=tb_s, scale=1.0)
        else:
            # tensor_tensor with a broadcast bias: DVE op form that carries
            # the round-to-f32r flag the BIR verifier requires of producers
            # feeding f32r matmuls (tensor_scalar does not)
            nc.vector.tensor_tensor(
                theta_t[b][:, ms].rearrange("p (a m) -> p a m", a=1),
                tps.rearrange("p (a m) -> p a m", a=1),
                tb_s[:, :, None].to_broadcast((128, 1, MC)),
                ALU.add,
            )

    def build_gt(b):
        """fill gT tile columns 0:128 by PE-transposing the pooled g."""
        gt = gt_t[b]
        g_flat = g_t[b].rearrange("p a b -> p (a b)")
        for half in range(2):
            gtps = pbank.tile([128, 4, 128], BF16, tag="bank", name="gtps")
            for j in range(4):
                pch = half * 4 + j
                nc.tensor.transpose(
                    gtps[:, j, :], g_flat[:, pch * 128:(pch + 1) * 128], identb_s,
                )
            nc.vector.tensor_copy(out=gt[:, half * 4:(half + 1) * 4, 0:128], in_=gtps)

    # ---------------- attention steps, software-pipelined ----------------
    fexp_t = {}

    def emit_f(b, mc):
        """f matmuls (fT layout) + exp on ACT -> fexp (128, 8, 512) bf16."""
        ms = slice(mc * MC, (mc + 1) * MC)
        fexp = fxp.tile([128, 8, MC], BF16, tag="fexp", name=f"fexp_{b}_{mc}")
        fexp_t[(b, mc)] = fexp
        phi_flat = phi_t[b].rearrange("p a b -> p (a b)")
        for half in range(4):
            fps = psf.tile([128, 2, MC], F32, tag="f")
            for i in range(2):
                pch = half * 2 + i
                nc.tensor.matmul(
                    fps[:, i, :],
                    phi_flat[:, pch * 128:(pch + 1) * 128],
                    theta_t[b][:, ms],
                    start=True, stop=True,
                )
            nc.scalar.activation(fexp[:, 2 * half:2 * half + 2, :], fps, ACT.Exp)

    def emit_z(b, mc):
        """z matmuls + normalize + transpose to zT (bf16) + Gram + row-sums.

        Both z matmul groups are emitted before the transposes/Gram so the
        PE never sits behind a transpose that waits on the DVE normalize.
        """
        nonlocal n_zz
        ms = slice(mc * MC, (mc + 1) * MC)
        fexp = fexp_t.pop((b, mc))
        gt = gt_t[b]
        tp = pbank.tile([128, 4, 128], BF16, tag="bank", name="tp")
        zbs = []
        for j2 in range(2):
            zb = pbank.tile([128, 512], F32, tag="bank", name="zb")
            zbs.append(zb)
            for i in range(2):
                sub = j2 * 2 + i
                for pch in range(8):
                    nc.tensor.matmul(
                        zb[:, i * 256:i * 256 + 132],
                        fexp[:, pch, sub * 128:(sub + 1) * 128],
                        gt[:, pch, :],
                        start=(pch == 0), stop=(pch == 7),
                    )
        zns = []
        for zb in zbs:
            zb2 = zb.rearrange("p (i c) -> p i c", i=2)
            rc = ptmp.tile([128, 2], F32, tag="rc", bufs=4)
            nc.vector.reciprocal(rc, zb2[:, :, 128])
            zn2 = znp.tile([128, 2, 128], BF16, tag="zn")
            nc.vector.tensor_tensor(
                zn2, zb2[:, :, 0:128],
                rc[:, :, None].to_broadcast((128, 2, 128)), ALU.mult,
            )
            zns.append(zn2)
        # transposes back-to-back (shared identity Ldweights), then Gram
        for j2, zn2 in enumerate(zns):
            for i in range(2):
                nc.tensor.transpose(tp[:, j2 * 2 + i, :], zn2[:, i, :], identb_s)
        for zn2 in zns:
            for i in range(2):
                # Gram accumulation for BN variance: ZZ += z_m^T z_m
                nc.tensor.matmul(
                    zz_ps, zn2[:, i, :], zn2[:, i, :],
                    start=(n_zz == 0), stop=(n_zz == zz_last),
                    skip_group_check=True,
                )
                n_zz += 1
        # PSUM -> SBUF zT copy; accum_out gives per-ci row sums over this mc
        idx = b * NMC + mc
        nc.vector.tensor_scalar(
            zt_t[b][:, ms], tp.rearrange("p a b -> p (a b)"), 1.0, 0.0,
            ALU.mult, ALU.add, accum_out=zsacc[:, idx:idx + 1],
        )

    # hook work interleaved into batch-0's attention steps
    hooks = {}
    for j in range(6):                       # b0 theta pieces 2..7
        hooks.setdefault((0, j), []).append(
            lambda b=0, mc=j + 2, oa=(2 <= j <= 5): theta_piece(b, mc, on_act=oa))
    for c in range(8):                       # b1 convs at steps 2..5
        hooks.setdefault((0, c // 2 + 2), []).append(lambda c=c: conv_piece(1, c))
    hooks.setdefault((0, 6), []).append(lambda: build_gt(1))
    hooks.setdefault((0, 6), []).append(lambda: theta_piece(1, 0))
    hooks.setdefault((0, 6), []).append(lambda: theta_piece(1, 1))
    hooks.setdefault((0, 7), []).append(lambda: theta_piece(1, 2))
    hooks.setdefault((0, 7), []).append(lambda: theta_piece(1, 3))
    for j in range(4):                       # b1 theta pieces 4..7
        hooks.setdefault((1, j), []).append(lambda mc=j + 4: theta_piece(1, mc))

    # PE p-state warm-up: the PE ramps to full clock only after ~4.5us of
    # continuous execution (one-time). Burn the ramp on junk matmuls over the
    # already-resident weight pack while the input DMAs stream.
    for w in range(6):
        wu = psf.tile([128, 2, MC], F32, tag="f", name="warm")
        nc.tensor.matmul(wu[:, 0, :], tw_s[:, 0, :], wp_s[:, 0:MC],
                         start=True, stop=True, skip_group_check=True)
        nc.tensor.matmul(wu[:, 1, :], tw_s[:, 0, :], wp_s[:, MC:2 * MC],
                         start=True, stop=True, skip_group_check=True)

    # batch-0 convs (DMA-gated startup) + first theta pieces; gt build last
    # (z(0,0) needs it ~4us after f(0,0), theta/phi gate f(0,0) itself)
    conv_piece(0, 0)
    conv_piece(0, 1)
    theta_piece(0, 0)
    for mc in range(2, NMC):
        conv_piece(0, mc)
    theta_piece(0, 1)
    build_gt(0)

    steps = [(0, mc) for mc in range(NMC)] + [(1, mc) for mc in range(NMC)]
    emit_f(*steps[0])
    for t, (b, mc) in enumerate(steps):
        if t + 1 < len(steps):
            emit_f(*steps[t + 1])
        for fn in hooks.get((b, mc), ()):
            fn()
        emit_z(b, mc)

    # ---------------- BN stats from z-statistics + AllReduce ----------------
    # ls[:, cc] = sum_n wz = W_cc . zsum ; ls[:, 2+cc] = sum_n wz^2 (Gram)
    ls = consts.tile([128, 4], F32)
    # var = E[wz^2] - mean^2 cancels catastrophically here (z columns share a
    # large common mean: mean^2 is up to 99% of E[wz^2], a ~170x error
    # amplifier), so the stats path needs two properties:
    #  - CONSISTENCY: measure moments of exactly the wz the tail computes,
    #    i.e. every W factor is the same bf16-rounded W (wwb/wrawb).
    #  - PRECISION: ZZ and zsum enter bf16 matmuls as hi/lo bf16 splits
    #    (two accumulated matmuls ~= a 16-bit mantissa, rel err ~2e-5).
    wrawb = consts.tile([128, 2, 128], BF16)
    nc.vector.tensor_copy(out=wrawb, in_=wraw_s)
    zz_hi = consts.tile([128, 128], BF16)
    nc.vector.tensor_copy(out=zz_hi, in_=zz_ps)
    zz_lo = consts.tile([128, 128], BF16)
    nc.vector.tensor_sub(zz_lo, zz_ps, zz_hi)
    zsum = consts.tile([128, 1], F32)
    nc.vector.reduce_sum(out=zsum, in_=zsacc, axis=AX.X)
    zs_hi = consts.tile([128, 1], BF16)
    nc.vector.tensor_copy(out=zs_hi, in_=zsum)
    zs_lo = consts.tile([128, 1], BF16)
    nc.vector.tensor_sub(zs_lo, zsum, zs_hi)
    mps = pbank.tile([128, 2], F32, tag="bank", name="mps")
    for cc in range(2):
        wc = wwb_s[:, cc * 128:(cc + 1) * 128]
        nc.tensor.matmul(mps[:, cc:cc + 1], wc, zs_hi, start=True, stop=False,
                         skip_group_check=True)
        nc.tensor.matmul(mps[:, cc:cc + 1], wc, zs_lo, start=False, stop=True,
                         skip_group_check=True)
        u_ps = pbank.tile([128, 128], F32, tag="bank", name="u_ps")
        nc.tensor.matmul(u_ps, wc, zz_hi, start=True, stop=False)
        nc.tensor.matmul(u_ps, wc, zz_lo, start=False, stop=True)
        qjunk = ptmp.tile([128, 128], F32, tag="qjunk", bufs=1)
        nc.vector.scalar_tensor_tensor(
            qjunk, u_ps, 1.0, wrawb[:, cc, :], ALU.mult, ALU.mult,
            accum_out=ls[:, 2 + cc:3 + cc],
        )
    nc.vector.tensor_copy(out=ls[:, 0:2], in_=mps)

    cc_in = dram.tile([128, 4], F32)
    cc_out = dram.tile([128, 4], F32)
    small_dma(cc_in, ls)
    if io.get("single_core_sim"):
        # stand-in for the AllReduce so TimelineSim (single-core) can run
        small_dma(cc_out, cc_in)
    else:
        nc.gpsimd.collective_compute(
            "AllReduce", ALU.add,
            replica_groups=[list(range(NCORES))],
            ins=[cc_in.opt()], outs=[cc_out.opt()],
        )
    gs = consts.tile([128, 4], F32)
    small_dma(gs, cc_out)

    inv = 1.0 / COUNT
    me2 = consts.tile([128, 4], F32)          # [mean(2) | E[wz^2](2)]
    nc.vector.tensor_scalar(me2, gs, inv, None, ALU.mult)
    mean = me2[:, 0:2]
    u = consts.tile([128, 2], F32)
    nc.vector.tensor_tensor(u, mean, mean, ALU.mult)
    nc.vector.tensor_sub(u, me2[:, 2:4], u)
    nc.vector.tensor_scalar(u, u, EPS, None, ALU.add)
    # table-free rsqrt on DVE (quake seed + 2 Newton steps). The Ln/Exp pair
    # used before forced two LoadActFuncSet round-trips (~2.6us) onto the
    # post-AllReduce critical path; this chain never touches the ACT tables.
    I32 = mybir.dt.int32
    USE_QUAKE = True
    if USE_QUAKE:
        t1 = consts.tile([128, 2], I32)
        nc.vector.tensor_scalar(t1, u.bitcast(I32), 1, None, ALU.logical_shift_right)
        t2 = consts.tile([128, 2], I32)
        nc.vector.tensor_scalar(t2, t1, -1, 0x5F3759DF, ALU.mult, ALU.add)
        r0 = t2.bitcast(F32)
    else:
        y0 = consts.tile([128, 2], F32)
        nc.scalar.activation(y0, u, ACT.Ln)
        r0 = consts.tile([128, 2], F32)
        nc.scalar.activation(r0, y0, ACT.Exp, scale=-0.5)
    for it in range(2):
        uy2 = consts.tile([128, 2], F32, name=f"uy2_{it}")
        nc.vector.tensor_mul(uy2, r0, r0)
        nc.vector.tensor_mul(uy2, uy2, u)
        half3 = consts.tile([128, 2], F32, name=f"half3_{it}")
        nc.vector.tensor_scalar(half3, uy2, -0.5, 1.5, ALU.mult, ALU.add)
        r1 = consts.tile([128, 2], F32, name=f"rs_{it}")
        nc.vector.tensor_mul(r1, r0, half3)
        r0 = r1
    a_s = consts.tile([128, 2], F32)
    nc.vector.tensor_mul(a_s, r0, gamma_s)
    nb = consts.tile([128, 2], F32)
    nc.vector.tensor_mul(nb, mean, a_s)
    nc.vector.tensor_sub(nb, beta_s, nb)

    # ---------------- fused tail: W conv + normalize + residual + store ----
    for b in range(BLOC):
        for cc in range(2):
            csl = slice(cc * 128, (cc + 1) * 128)
            for mc in range(NMC):
                ms = slice(mc * MC, (mc + 1) * MC)
                k = (b * 2 + cc) * NMC + mc
                wpool, wtag = (pbank, "bank") if k % 2 == 0 else (psf, "f")
                wb = wpool.tile([128, MC], F32, tag=wtag, name="wb")
                nc.tensor.matmul(wb, wwb_s[:, cc * 128:(cc + 1) * 128],
                                 zt_t[b][:, ms], start=True, stop=True)
                wn = outp.tile([128, MC], F32, tag="wn")
                nc.scalar.activation(wn, wb, ACT.Identity,
                                     bias=nb[:, cc:cc + 1], scale=a_s[:, cc:cc + 1])
                # residual in place into the resident x tile, DMA straight
                # out. The add writes through the F32R view: x_t is also read
                # by the (long done) f32r theta matmuls, and the BIR verifier
                # statically requires every writer of an f32r-matmul input to
                # round. Costs ~2.4e-4 relative rounding on the output.
                xr = x_tiles[b][:, cc, ms]
                xv = xr.bitcast(F32)
                if k % 3 == 2:
                    nc.gpsimd.tensor_add(xr, xv, wn)
                else:
                    nc.vector.tensor_add(xr, xv, wn)
                nc.sync.dma_start(out=out[b, csl, ms], in_=xv)


_CACHE = {}


def make_io(nc):
    return {
        "x": nc.dram_tensor("x", [BLOC, C, N], F32R, kind="ExternalInput").ap(),
        "y": nc.dram_tensor("y", [BLOC, C, N], F32R, kind="ExternalInput").ap(),
        "wpack": nc.dram_tensor("wpack", [128, 1472], F32R, kind="ExternalInput").ap(),
        "vpack": nc.dram_tensor("vpack", [128, 5], F32, kind="ExternalInput").ap(),
        "gpad": nc.dram_tensor("gpad", [128, 8, 4], BF16, kind="ExternalInput").ap(),
        "out": nc.dram_tensor("out", [BLOC, C, N], F32, kind="ExternalOutput").ap(),
    }


def _get_program():
    if "nc" in _CACHE:
        return _CACHE["nc"], _CACHE["io"]
    nc = bacc.Bacc(
        "TRN2", target_bir_lowering=False, debug=False,
        enable_asserts=False, num_devices=NCORES,
    )
    io = make_io(nc)
    from contextlib import ExitStack
    with tile.TileContext(nc) as tc:
        with ExitStack() as ctx:
            io["ctx"] = ctx
            build_body(tc, io)
    nc.compile()
    _CACHE["nc"] = nc
    _CACHE["io"] = io
    return nc, io


def kernel(x, y, theta_w, theta_b, phi_w, phi_b, g_w, g_b, W_w, W_b,
           bn_gamma, bn_beta, _trace=False, **_unused):
    x = np.asarray(x, dtype=np.float32).reshape(B, C, N)
    y = np.asarray(y, dtype=np.float32).reshape(B, C, N)

    def chunked(wT):
        # (C, CI) -> (128, 2, CI): [p, k, ci] = wT[k*128+p, ci]
        return np.asarray(wT, np.float32).reshape(2, 128, CI).transpose(1, 0, 2)

    tw = chunked(np.asarray(theta_w, np.float32).T)
    pw = chunked(np.asarray(phi_w, np.float32).T)
    gw = chunked(np.asarray(g_w, np.float32).T)
    ww = np.asarray(W_w, np.float32).T                             # (CI, C)
    wraw = chunked(np.asarray(W_w, np.float32))                    # c-part layout
    ident = np.eye(128, dtype=np.float32)
    # bf16 identity packed as raw bits into 64 f32 columns of wpack
    eye_bits = np.eye(128, dtype=np.float32).astype(ml_bf16).view(np.uint16).astype(np.uint32)
    packed = (eye_bits[:, 1::2] << 16) | eye_bits[:, 0::2]
    wpack = np.ascontiguousarray(np.concatenate([
        tw.reshape(128, 256), pw.reshape(128, 256), gw.reshape(128, 256),
        ww, ident, wraw.reshape(128, 256),
        packed.view(np.float32)], axis=1))
    tb = np.asarray(theta_b, np.float32).reshape(CI, 1)
    gamma = np.asarray(bn_gamma, np.float32).reshape(2, 128).T
    beta = np.asarray(bn_beta, np.float32).reshape(2, 128).T
    vpack = np.ascontiguousarray(np.concatenate([tb, gamma, beta], axis=1))
    gpad = np.zeros((128, 8, 4), ml_bf16)
    gpad[:, :, 0] = 1.0
    # phi_b, g_b, W_b intentionally unused: softmax-invariant / cancelled by BN.

    nc, _ = _get_program()
    in_maps = []
    for k in range(NCORES):
        in_maps.append({
            "x": np.ascontiguousarray(x[k * BLOC:(k + 1) * BLOC]),
            "y": np.ascontiguousarray(y[k * BLOC:(k + 1) * BLOC]),
            "wpack": wpack, "vpack": vpack, "gpad": gpad,
        })
    res = run_bass_kernel_spmd(nc, in_maps, core_ids=list(range(NCORES)), trace=_trace)
    out = np.concatenate([r_["out"] for r_ in res.results], axis=0)
    if _trace:
        _CACHE["last_results"] = res
    return out.reshape(B, C, 64, 64)
